# revision 8
# baseline (speedup 1.0000x reference)
"""Trainium2 Bass kernel for nn_Condition9RotR.

Pipeline (per row r of N=200000):
    h1 = relu(feature @ W1 + b1); h2 = relu(h1 @ W2 + b2)
    mat = (h2 @ W3 + b3).reshape(3,3) + I
    A = rotation @ mat                      (batched 3x3)
    rot_out = polar(A) = U @ Vh from SVD    (orthogonal polar factor)
    logdet = 0

Device strategy (8 cores, pure data parallel over rows):
  * Host pads N -> 200704 = 8*25088 rows, pre-transposes feature so the
    kernel streams X^T directly (contraction dim on partitions).
  * MLP runs "transposed": h1T/h2T/out3T have the hidden dim on
    partitions and rows on the matmul free dim, so biases are
    per-partition and fuse into the ScalarE relu that evacuates PSUM.
  * out3T [9, rows] is scattered (SBUF->SBUF DMA) into a component-major
    layout [128 partitions, 9 * F] (F rows per partition).
  * polar(A) is computed with Newton's iteration for the polar factor,
    X <- (mu*X + (1/mu)*cof(X)/det(X)) / 2 (cof = signed cofactors, so
    cof(X)/det(X) = X^-T), 3 determinant-scaled + 5 plain iterations.
    All 3x3 algebra is done with fused multi-component DVE access
    patterns on [128, 9*F] tiles.
"""

import numpy as np

import concourse.bass as bass
import concourse.bacc as bacc
import concourse.mybir as mybir
import concourse.tile as tile
from concourse.bass_utils import run_bass_kernel_spmd

F32 = mybir.dt.float32
F32R = mybir.dt.float32r
MM_CAST = False       # float32r needs producer-side rounding; BIR verifier rejects bitcast
AF = mybir.ActivationFunctionType
OP = mybir.AluOpType

N = 200000
NCORES = 8
R = 25088              # rows per core (padded): 8*25088 = 200704
NPAD = NCORES * R
FEAT = 512
HID = 256

F = 98                 # rows per partition per superchunk
NSC = 2                # superchunks per core (each 128*F = 12544 rows)
SCROWS = 128 * F
NSLAB = 4              # scatter slabs per superchunk (32 partitions each)
SLAB = 32 * F          # 3136 rows
NMM = 448              # rows per matmul instruction (<=512 fp32)
CPS = SLAB // NMM      # 7 chunks per slab

N_SCALED = 3           # det-scaled Newton iterations
N_PLAIN = 5            # plain Newton iterations

EYE9 = np.eye(3, dtype=np.float32).reshape(9)


def _mm(ap):
    return ap.bitcast(F32R) if MM_CAST else ap


def _apx(buf, off, dims):
    """AP on `buf`'s tensor at free-offset `off` with explicit free dims
    [[step, count], ...]; partition dim copied from buf."""
    return bass.AP(buf.tensor, buf.offset + off, [list(buf.ap[0])] + dims)


def _cofactor(nc, X, C, P, f):
    """C = signed cofactor matrix of X (so X^-T = C / det(X)).

    X, C, P: [128, 9*f] tiles, layout comp-major (i*3+j)*f + t.
    P is scratch.  Emits 9 DVE tensor_tensor ops using rectangle APs that
    cover the mod-3 index pattern with arithmetic strides.
    """
    mul = OP.mult

    def rects(dst, swap, eng=None):
        # P1: dst[i,j] = X[i+1, j+1] * X[i+2, j+2]   (indices mod 3)
        # P2 (swap=True): dst[i,j] = X[i+1, j+2] * X[i+2, j+1]
        # rect A: i in {0,1}, j in {0,1}
        if not swap:
            a = _apx(X, 4 * f, [[3 * f, 2], [f, 2], [1, f]])      # X[i+1,j+1]
            b = _apx(X, 8 * f, [[-6 * f, 2], [-2 * f, 2], [1, f]])  # X[i+2,j+2]
        else:
            a = _apx(X, 5 * f, [[3 * f, 2], [-2 * f, 2], [1, f]])  # X[i+1,j+2]
            b = _apx(X, 7 * f, [[-6 * f, 2], [f, 2], [1, f]])      # X[i+2,j+1]
        o = _apx(dst, 0, [[3 * f, 2], [f, 2], [1, f]])
        eng.tensor_tensor(o, a, b, mul)
        # rect B: i = 2, j in {0,1}
        if not swap:
            a = _apx(X, 1 * f, [[f, 2], [1, f]])                   # X[0,j+1]
            b = _apx(X, 5 * f, [[-2 * f, 2], [1, f]])              # X[1,j+2]
        else:
            a = _apx(X, 2 * f, [[-2 * f, 2], [1, f]])              # X[0,j+2]
            b = _apx(X, 4 * f, [[f, 2], [1, f]])                   # X[1,j+1]
        o = _apx(dst, 6 * f, [[f, 2], [1, f]])
        eng.tensor_tensor(o, a, b, mul)
        # rect C: i in {0,1}, j = 2
        if not swap:
            a = _apx(X, 3 * f, [[3 * f, 2], [1, f]])               # X[i+1,0]
            b = _apx(X, 7 * f, [[-6 * f, 2], [1, f]])              # X[i+2,1]
        else:
            a = _apx(X, 4 * f, [[3 * f, 2], [1, f]])               # X[i+1,1]
            b = _apx(X, 6 * f, [[-6 * f, 2], [1, f]])              # X[i+2,0]
        o = _apx(dst, 2 * f, [[3 * f, 2], [1, f]])
        eng.tensor_tensor(o, a, b, mul)
        # rect D: i = 2, j = 2
        if not swap:
            a = _apx(X, 0, [[1, f]])                               # X[0,0]
            b = _apx(X, 4 * f, [[1, f]])                           # X[1,1]
        else:
            a = _apx(X, 1 * f, [[1, f]])                           # X[0,1]
            b = _apx(X, 3 * f, [[1, f]])                           # X[1,0]
        o = _apx(dst, 8 * f, [[1, f]])
        eng.tensor_tensor(o, a, b, mul)

    rects(C, swap=False, eng=nc.vector)   # P1 -> C
    rects(P, swap=True, eng=nc.vector)    # P2 -> P
    nc.vector.tensor_tensor(C[:, : 9 * f], C[:, : 9 * f], P[:, : 9 * f],
                            OP.subtract)


def build_program():
    nc = bacc.Bacc("TRN2", target_bir_lowering=False, debug=False,
                   num_devices=NCORES)

    xtd = nc.declare_dram_parameter("xt", [FEAT, R], F32, isOutput=False)
    rotd = nc.declare_dram_parameter("rot", [R, 9], F32, isOutput=False)
    w1d = nc.declare_dram_parameter("w1", [FEAT, HID], F32, isOutput=False)
    w2d = nc.declare_dram_parameter("w2", [HID, HID], F32, isOutput=False)
    w3d = nc.declare_dram_parameter("w3", [HID, 9], F32, isOutput=False)
    b1d = nc.declare_dram_parameter("b1", [HID], F32, isOutput=False)
    b2d = nc.declare_dram_parameter("b2", [HID], F32, isOutput=False)
    b3ed = nc.declare_dram_parameter("b3e", [9], F32, isOutput=False)
    outd = nc.declare_dram_parameter("out", [R, 9], F32, isOutput=True)

    xtd_r = xtd.rearrange("(k p) r -> p k r", k=4)                  # [128,4,R]
    rotd_f = rotd.rearrange("(s q t) j -> s q (t j)", s=NSC, q=128)  # [2,128,882]
    outd_f = outd.rearrange("(s q t) j -> s q (t j)", s=NSC, q=128)

    with tile.TileContext(nc) as tc:
        with (
            tc.tile_pool(name="const", bufs=1) as cpool,
            tc.tile_pool(name="mlpx", bufs=3) as xpool,
            tc.tile_pool(name="hact", bufs=2) as hpool,
            tc.tile_pool(name="o3p", bufs=2) as opool,
            tc.tile_pool(name="dense", bufs=2) as dpool,
            tc.tile_pool(name="ps12", bufs=3, space="PSUM") as pspool,
            tc.tile_pool(name="ps3p", bufs=2, space="PSUM") as ps3pool,
        ):
            # ---- constants (weights / biases) ----
            w1_sb = cpool.tile([128, 4 * HID], F32, name="w1_sb")
            nc.sync.dma_start(
                out=w1_sb.rearrange("p (k j) -> p k j", k=4),
                in_=w1d.rearrange("(k p) j -> p k j", k=4),
            )
            w2_sb = cpool.tile([128, 2 * HID], F32, name="w2_sb")
            nc.sync.dma_start(
                out=w2_sb.rearrange("p (k j) -> p k j", k=2),
                in_=w2d.rearrange("(k p) j -> p k j", k=2),
            )
            w3_sb = cpool.tile([128, 2 * 9], F32, name="w3_sb")
            nc.sync.dma_start(
                out=w3_sb.rearrange("p (k j) -> p k j", k=2),
                in_=w3d.rearrange("(k p) j -> p k j", k=2),
            )
            b1_sb = cpool.tile([128, 2], F32, name="b1_sb")
            nc.sync.dma_start(out=b1_sb, in_=b1d.rearrange("(m p) -> p m", m=2))
            b2_sb = cpool.tile([128, 2], F32, name="b2_sb")
            nc.sync.dma_start(out=b2_sb, in_=b2d.rearrange("(m p) -> p m", m=2))
            b3e_sb = cpool.tile([9, 1], F32, name="b3e_sb")
            nc.sync.dma_start(out=b3e_sb, in_=b3ed.rearrange("(j o) -> j o", o=1))

            w1v = w1_sb.rearrange("p (k m j) -> p k m j", k=4, m=2)
            w2v = w2_sb.rearrange("p (k m j) -> p k m j", k=2, m=2)
            w3v = w3_sb.rearrange("p (k j) -> p k j", k=2)

            for s in range(NSC):
                rot_lin = dpool.tile([128, 9 * F], F32, tag="rot_lin",
                                     name="rot_lin")
                nc.sync.dma_start(out=rot_lin, in_=rotd_f[s])
                matd = dpool.tile([128, 9 * F], F32, tag="matd", name="matd")

                # ---------- MLP over this superchunk ----------
                for v in range(NSLAB):
                    o3 = opool.tile([9, SLAB], F32, tag="o3", name="o3")
                    for cc in range(CPS):
                        c = (s * NSLAB + v) * CPS + cc
                        xt_t = xpool.tile([128, 4 * NMM], F32, tag="xt",
                                          name="xt_t")
                        xt3 = xt_t.rearrange("p (k r) -> p k r", k=4)
                        nc.sync.dma_start(
                            out=xt3, in_=xtd_r[:, :, c * NMM:(c + 1) * NMM])
                        h1 = hpool.tile([128, 2 * NMM], F32, tag="h1", name="h1")
                        for m in range(2):
                            ps1 = pspool.tile([128, NMM], F32, tag="ps1",
                                              name="ps1")
                            for k in range(4):
                                nc.tensor.matmul(
                                    ps1, lhsT=_mm(w1v[:, k, m, :]),
                                    rhs=_mm(xt3[:, k, :]),
                                    start=(k == 0), stop=(k == 3))
                            nc.scalar.activation(
                                out=h1[:, m * NMM:(m + 1) * NMM], in_=ps1,
                                func=AF.Relu, bias=b1_sb[:, m:m + 1])
                        h2 = hpool.tile([128, 2 * NMM], F32, tag="h2", name="h2")
                        for m in range(2):
                            ps2 = pspool.tile([128, NMM], F32, tag="ps2",
                                              name="ps2")
                            for k in range(2):
                                nc.tensor.matmul(
                                    ps2, lhsT=_mm(w2v[:, k, m, :]),
                                    rhs=_mm(h1[:, k * NMM:(k + 1) * NMM]),
                                    start=(k == 0), stop=(k == 1))
                            nc.scalar.activation(
                                out=h2[:, m * NMM:(m + 1) * NMM], in_=ps2,
                                func=AF.Relu, bias=b2_sb[:, m:m + 1])
                        ps3 = ps3pool.tile([9, NMM], F32, tag="ps3", name="ps3")
                        for k in range(2):
                            nc.tensor.matmul(
                                ps3, lhsT=_mm(w3v[:, k, :]),
                                rhs=_mm(h2[:, k * NMM:(k + 1) * NMM]),
                                start=(k == 0), stop=(k == 1))
                        # out3 + (b3 + I) -> out3T slab
                        nc.scalar.activation(
                            out=o3[:, cc * NMM:(cc + 1) * NMM], in_=ps3,
                            func=AF.Identity, bias=b3e_sb[:, 0:1])
                    # scatter slab into component-major matd
                    for j in range(9):
                        # component j of out3T; rows r -> (q=r//F, t=r%F)
                        src = bass.AP(o3.tensor, o3.offset + j * o3.ap[0][0],
                                      [[o3.ap[0][0], 1], [F, 32], [1, F]])
                        dst = matd[32 * v:32 * (v + 1), j * F:(j + 1) * F]
                        nc.gpsimd.dma_start(out=dst, in_=src)

                # ---------- dense 3x3 stage for this superchunk ----------
                rote = dpool.tile([128, 9 * F], F32, tag="rote", name="rote")
                rlv = rot_lin.rearrange("p (t j) -> p j t", j=9)
                for j in range(9):
                    nc.gpsimd.tensor_copy(rote[:, j * F:(j + 1) * F],
                                          rlv[:, j, :])

                xa = dpool.tile([128, 9 * F], F32, tag="xa", name="xa")
                xb = dpool.tile([128, 9 * F], F32, tag="xb", name="xb")
                cb = dpool.tile([128, 9 * F], F32, tag="cb", name="cb")
                pb = dpool.tile([128, 9 * F], F32, tag="pb", name="pb")
                ub = dpool.tile([128, 9 * F], F32, tag="ub", name="ub")
                det = dpool.tile([128, F], F32, tag="det", name="det")
                mu = dpool.tile([128, F], F32, tag="mu", name="mu")
                qq = dpool.tile([128, F], F32, tag="qq", name="qq")
                sc = dpool.tile([128, F], F32, tag="sc", name="sc")
                scr = dpool.tile([128, F], F32, tag="scr", name="scr")

                # X0 = rot @ mat  (batched 3x3: out[i,k] = sum_j r[i,j]m[j,k])
                r3 = rote.rearrange("p (i j t) -> p i j t", i=3, j=3)
                m3 = matd.rearrange("p (j k t) -> p j k t", j=3, k=3)

                def ikt(buf):
                    return buf.rearrange("p (i k t) -> p i k t", i=3, k=3)

                def jprod(dst, j, eng):
                    a = r3[:, :, j, :].unsqueeze(2).broadcast_to([128, 3, 3, F])
                    b = m3[:, j, :, :].unsqueeze(1).broadcast_to([128, 3, 3, F])
                    eng.tensor_tensor(ikt(dst), a, b, OP.mult)

                jprod(cb, 0, nc.vector)
                jprod(pb, 1, nc.vector)
                nc.vector.tensor_tensor(xa[:, :], cb[:, :], pb[:, :], OP.add)
                jprod(ub, 2, nc.vector)
                nc.vector.tensor_tensor(xa[:, :], xa[:, :], ub[:, :], OP.add)

                # ---- Newton polar iterations ----
                X, Xo = xa, xb
                for it in range(N_SCALED + N_PLAIN):
                    _cofactor(nc, X, cb, pb, F)   # cof -> cb (pb scratch)
                    # det = sum_j X[0,j] * C[0,j]
                    nc.vector.tensor_tensor(pb[:, :3 * F], X[:, :3 * F],
                                            cb[:, :3 * F], OP.mult)
                    nc.vector.tensor_reduce(
                        out=det[:, :],
                        in_=pb.rearrange("p (j t) -> p t j", j=9)[:, :, :3],
                        axis=mybir.AxisListType.X, op=OP.add)
                    if it < N_SCALED:
                        # mu = |det|^(-1/3) via exp(-ln(|det|)/3)
                        nc.scalar.activation(out=mu, in_=det, func=AF.Abs)
                        nc.scalar.activation(out=qq, in_=mu, func=AF.Ln)
                        nc.scalar.activation(out=mu, in_=qq, func=AF.Exp,
                                             scale=-1.0 / 3.0)
                        # s = 1 / (mu * det)
                        nc.vector.tensor_tensor(qq, det, mu, OP.mult)
                        nc.vector.reciprocal_approx_accurate(
                            out=sc, in_=qq, scratch=scr)
                        mu_b = mu.unsqueeze(1).broadcast_to([128, 9, F])
                        s_b = sc.unsqueeze(1).broadcast_to([128, 9, F])
                        x9 = X.rearrange("p (c t) -> p c t", c=9)
                        c9 = cb.rearrange("p (c t) -> p c t", c=9)
                        u9 = ub.rearrange("p (c t) -> p c t", c=9)
                        p9 = pb.rearrange("p (c t) -> p c t", c=9)
                        # u = 0.5 * mu * X ; p = 0.5 * s * C ; Xo = u + p
                        nc.vector.scalar_tensor_tensor(
                            out=u9, in0=x9, scalar=0.5, in1=mu_b,
                            op0=OP.mult, op1=OP.mult)
                        nc.vector.scalar_tensor_tensor(
                            out=p9, in0=c9, scalar=0.5, in1=s_b,
                            op0=OP.mult, op1=OP.mult)
                        nc.vector.tensor_tensor(Xo[:, :], ub[:, :], pb[:, :],
                                                OP.add)
                    else:
                        nc.vector.reciprocal_approx_accurate(
                            out=sc, in_=det, scratch=scr)
                        s_b = sc.unsqueeze(1).broadcast_to([128, 9, F])
                        c9 = cb.rearrange("p (c t) -> p c t", c=9)
                        x9 = X.rearrange("p (c t) -> p c t", c=9)
                        p9 = pb.rearrange("p (c t) -> p c t", c=9)
                        xo9 = Xo.rearrange("p (c t) -> p c t", c=9)
                        # p = 0.5 * s * C ; Xo = 0.5 * X + p
                        nc.vector.scalar_tensor_tensor(
                            out=p9, in0=c9, scalar=0.5, in1=s_b,
                            op0=OP.mult, op1=OP.mult)
                        nc.vector.scalar_tensor_tensor(
                            out=xo9, in0=x9, scalar=0.5, in1=p9,
                            op0=OP.mult, op1=OP.add)
                    X, Xo = Xo, X

                # ---- interleave result and store ----
                out_lin = dpool.tile([128, 9 * F], F32, tag="out_lin",
                                     name="out_lin")
                olv = out_lin.rearrange("p (t j) -> p j t", j=9)
                for j in range(9):
                    nc.gpsimd.tensor_copy(olv[:, j, :],
                                          X[:, j * F:(j + 1) * F])
                nc.sync.dma_start(out=outd_f[s], in_=out_lin)

    nc.compile()
    return nc


_NC = None


def _get_nc():
    global _NC
    if _NC is None:
        _NC = build_program()
    return _NC


def kernel(**inputs):
    rotation = np.ascontiguousarray(inputs["rotation"]).astype(
        np.float32, copy=False).reshape(-1, 9)
    feature = np.ascontiguousarray(inputs["feature"]).astype(
        np.float32, copy=False)
    w1 = np.ascontiguousarray(inputs["W1"]).astype(np.float32, copy=False)
    w2 = np.ascontiguousarray(inputs["W2"]).astype(np.float32, copy=False)
    w3 = np.ascontiguousarray(inputs["W3"]).astype(np.float32, copy=False)
    b1 = np.ascontiguousarray(inputs["b1"]).astype(np.float32, copy=False)
    b2 = np.ascontiguousarray(inputs["b2"]).astype(np.float32, copy=False)
    b3e = np.ascontiguousarray(inputs["b3"]).astype(
        np.float32, copy=False) + EYE9

    n = rotation.shape[0]
    pad = NPAD - n
    rot_pad = np.concatenate(
        [rotation, np.tile(EYE9[None, :], (pad, 1))], axis=0)
    feat_pad = np.concatenate(
        [feature, np.zeros((pad, FEAT), np.float32)], axis=0)

    nc = _get_nc()
    in_maps = []
    for i in range(NCORES):
        sl = slice(i * R, (i + 1) * R)
        in_maps.append({
            "xt": np.ascontiguousarray(feat_pad[sl].T),
            "rot": np.ascontiguousarray(rot_pad[sl]),
            "w1": w1, "w2": w2, "w3": w3,
            "b1": b1, "b2": b2, "b3e": b3e,
        })
    res = run_bass_kernel_spmd(nc, in_maps, list(range(NCORES)))
    out = np.concatenate([res.results[i]["out"] for i in range(NCORES)],
                         axis=0)[:n]
    return out.reshape(n, 3, 3), np.zeros((n,), np.float32)
